# revision 1
# baseline (speedup 1.0000x reference)
"""GCN message-passing kernel for Trainium2 (8 NeuronCores, SPMD).

out = (D^-1/2 (A+I) D^-1/2 X) W^T + b   for a random graph with
N=100000 nodes, E=1600000 edges, 128 channels.

Strategy (per core; destinations sharded 12500 nodes/core):
- Every edge becomes a "token" with coefficient gamma = dinv[src]*dinv[dst];
  aggregation for a window of 128 destinations is
  aggT[ch, dst] = sum_tokens x[src]^T * onehot,
  onehot[e, d] = gamma[e] * (destrel[e] == d), computed as fp32 matmuls
  accumulated in PSUM (lhsT = gathered message tile, rhs = one-hot tile).
- Gathers use dma_gather (SWDGE, int16 indices) against 4 banked views of x
  (32768 rows each); calls round-robin over 4 SWDGE queues so descriptor
  generation runs on all 8 GpSimd cores in parallel.
- Self-loops skip the gather: each window's own x rows load with a plain
  sequential DMA and enter the same one-hot matmul path.
- Token order: [super of 16 windows][source bank][window][128-token tiles],
  padded with gamma=0 tokens so all 8 cores share one instruction stream.
- Finalize per window: outT = W^T @ aggT + b, written as outT[128, 12544]
  per core; host transposes/assembles.
"""

import sys

sys.path.insert(0, "/opt/trn_rl_repo")
import numpy as np

N = 100000
D = 128
CORES = 8
NPC = N // CORES  # 12500 dests per core
NW = (NPC + 127) // 128  # 98 windows per core
SUP = 6  # windows per super (PSUM accumulator banks: 6 + 2 for out matmul)
NSUP = (NW + SUP - 1) // SUP  # 7 supers
BANK = 32768
NBANKS = (N + BANK - 1) // BANK  # 4
CHUNK = 16  # gather-call size in 128-token tiles


def _build_bass(tiles, sup_windows):
    """Build the SPMD Bass program.

    tiles: int array [NSUP, NBANKS, NW] - tile count per group (global).
    sup_windows: list of per-super window lists.
    """
    import os

    import concourse.mybir as mybir
    import concourse.tile as tile
    from concourse import bacc

    lim_sup = int(os.environ.get("K_LIMIT_SUPERS", NSUP))
    T_total = int(tiles.sum())
    NTOK = 128 * T_total

    nc = bacc.Bacc(None, target_bir_lowering=False, num_swdge_queues=4)
    xt = nc.dram_tensor("xt", [N, D], mybir.dt.float32, kind="ExternalInput")
    idxs = nc.dram_tensor("idxs", [128, NTOK // 16], mybir.dt.int16, kind="ExternalInput")
    destrel = nc.dram_tensor("destrel", [128, T_total], mybir.dt.float32, kind="ExternalInput")
    gamma = nc.dram_tensor("gamma", [128, T_total], mybir.dt.float32, kind="ExternalInput")
    gself = nc.dram_tensor("gself", [128, NW], mybir.dt.float32, kind="ExternalInput")
    wt = nc.dram_tensor("wt", [D, D], mybir.dt.float32, kind="ExternalInput")
    bvec = nc.dram_tensor("bvec", [D, 1], mybir.dt.float32, kind="ExternalInput")
    outT = nc.dram_tensor("outT", [D, NW * 128], mybir.dt.float32, kind="ExternalOutput")

    xviews = [xt[b * BANK : min((b + 1) * BANK, N), :] for b in range(NBANKS)]

    # per-core compact x rows for self-loop loads (sequential DMA)
    xself_in = nc.dram_tensor("xself", [NW * 128, D], mybir.dt.float32, kind="ExternalInput")

    gq = [0]
    OHK = 16  # one-hot batch, in tiles
    with tile.TileContext(nc) as tc:
        with (
            tc.tile_pool(name="const", bufs=1) as cpool,
            tc.tile_pool(name="meta", bufs=1) as mpool,
            tc.tile_pool(name="gp", bufs=8) as gpool,
            tc.tile_pool(name="ohp", bufs=4) as ohpool,
            tc.tile_pool(name="sohp", bufs=2) as sohpool,
            tc.tile_pool(name="selfp", bufs=3) as selfpool,
            tc.tile_pool(name="rhp", bufs=3) as rhpool,
            tc.tile_pool(name="outp", bufs=2) as outpool,
            tc.tile_pool(name="idxp", bufs=8) as idxpool,
            tc.tile_pool(name="ps", bufs=1, space="PSUM") as pspool,
            tc.tile_pool(name="pso", bufs=2, space="PSUM") as psopool,
        ):
            wt_t = cpool.tile([D, D], mybir.dt.float32)
            nc.sync.dma_start(out=wt_t[:], in_=wt[:])
            b_t = cpool.tile([D, 1], mybir.dt.float32)
            nc.sync.dma_start(out=b_t[:], in_=bvec[:])
            iota_t = cpool.tile([128, 128], mybir.dt.float32)
            nc.gpsimd.iota(
                iota_t[:], pattern=[[1, 128]], base=0, channel_multiplier=0,
                allow_small_or_imprecise_dtypes=True,
            )
            pidx_t = cpool.tile([128, 1], mybir.dt.float32)
            nc.gpsimd.iota(
                pidx_t[:], pattern=[[1, 1]], base=0, channel_multiplier=1,
                allow_small_or_imprecise_dtypes=True,
            )
            # constant self one-hot base: (iota == p)
            selfbase_t = cpool.tile([128, 128], mybir.dt.float32)
            nc.vector.tensor_tensor(
                out=selfbase_t[:],
                in0=iota_t[:],
                in1=pidx_t[:, 0:1].to_broadcast([128, 128]),
                op=mybir.AluOpType.is_equal,
            )
            destrel_t = mpool.tile([128, T_total], mybir.dt.float32)
            nc.sync.dma_start(out=destrel_t[:], in_=destrel[:])
            gamma_t = mpool.tile([128, T_total], mybir.dt.float32)
            nc.sync.dma_start(out=gamma_t[:], in_=gamma[:])
            gself_t = mpool.tile([128, NW], mybir.dt.float32)
            nc.sync.dma_start(out=gself_t[:], in_=gself[:])

            oh_batches = {}

            def oh_for(gt):
                bnum = gt // OHK
                if bnum not in oh_batches:
                    t0 = bnum * OHK
                    k = min(OHK, T_total - t0)
                    ohb = ohpool.tile([128, OHK, 128], mybir.dt.float32, tag="ohb")
                    nc.vector.tensor_tensor(
                        out=ohb[:, :k, :],
                        in0=iota_t[:, None, :].to_broadcast([128, k, 128]),
                        in1=destrel_t[:, t0 : t0 + k, None].to_broadcast([128, k, 128]),
                        op=mybir.AluOpType.is_equal,
                    )
                    nc.vector.tensor_tensor(
                        out=ohb[:, :k, :],
                        in0=ohb[:, :k, :],
                        in1=gamma_t[:, t0 : t0 + k, None].to_broadcast([128, k, 128]),
                        op=mybir.AluOpType.mult,
                    )
                    oh_batches[bnum] = ohb
                    for old in list(oh_batches):
                        if old < bnum - 2:
                            del oh_batches[old]
                return oh_batches[bnum][:, gt % OHK, :]

            tile_cursor = 0  # global tile index in token order
            for S in range(NSUP):
                if S >= lim_sup:
                    break
                wins = sup_windows[S]
                nwin = len(wins)
                # batched self one-hots for this super
                soh = sohpool.tile([128, SUP, 128], mybir.dt.float32, tag="soh")
                nc.vector.tensor_tensor(
                    out=soh[:, :nwin, :],
                    in0=selfbase_t[:, None, :].to_broadcast([128, nwin, 128]),
                    in1=gself_t[:, wins[0] : wins[0] + nwin, None].to_broadcast(
                        [128, nwin, 128]
                    ),
                    op=mybir.AluOpType.mult,
                )
                psbank = {}
                mm_done = {w: 0 for w in wins}
                mm_total = {
                    w: 1 + int(sum(tiles[S, bb, w] for bb in range(NBANKS)))
                    for w in wins
                }
                for b in range(NBANKS):
                    region_tiles = int(sum(tiles[S, b, w] for w in wins))
                    chunk_tiles = []  # (start_tile_global, len, sbuf_tile)
                    c0 = 0
                    while c0 < region_tiles:
                        clen = min(CHUNK, region_tiles - c0)
                        gtile = gpool.tile([128, CHUNK, D], mybir.dt.float32, tag="g")
                        itile = idxpool.tile([128, CHUNK * 8], mybir.dt.int16, tag="ix")
                        gstart = tile_cursor + c0
                        nc.sync.dma_start(
                            out=itile[:, : clen * 8],
                            in_=idxs[:, gstart * 8 : (gstart + clen) * 8],
                        )
                        nc.gpsimd.dma_gather(
                            gtile[:, :clen, :],
                            xviews[b],
                            itile[:, : clen * 8],
                            128 * clen,
                            128 * clen,
                            D,
                            elem_step=D,
                            single_packet=False,
                            queue_num=gq[0] % 4,
                        )
                        gq[0] += 1
                        chunk_tiles.append((gstart, clen, gtile))
                        c0 += clen

                    def chunk_for(gt):
                        for cs, cl, ct in chunk_tiles:
                            if cs <= gt < cs + cl:
                                return ct, gt - cs
                        raise AssertionError

                    t_local = tile_cursor
                    for wi, w in enumerate(wins):
                        nt = int(tiles[S, b, w])
                        if b == 0:
                            # self-loop tile first: sequential x rows
                            ps = pspool.tile(
                                [128, 128], mybir.dt.float32, tag=f"psw{w % SUP}",
                                name=f"psw{S}_{w % SUP}",
                            )
                            psbank[w] = ps
                            xs = selfpool.tile([128, D], mybir.dt.float32, tag="xs")
                            nc.sync.dma_start(
                                out=xs[:], in_=xself_in[w * 128 : (w + 1) * 128, :]
                            )
                            nc.tensor.matmul(
                                out=ps[:],
                                lhsT=xs[:],
                                rhs=soh[:, wi, :],
                                start=True,
                                stop=(mm_total[w] == 1),
                                skip_group_check=True,
                            )
                            mm_done[w] = 1
                        for t in range(nt):
                            gt = t_local + t
                            ctile, ccol = chunk_for(gt)
                            nc.tensor.matmul(
                                out=psbank[w][:],
                                lhsT=ctile[:, ccol, :],
                                rhs=oh_for(gt),
                                start=False,
                                stop=(mm_done[w] == mm_total[w] - 1),
                                skip_group_check=True,
                            )
                            mm_done[w] += 1
                        t_local += nt
                    tile_cursor += region_tiles

                # finalize super: outT_w = W^T @ aggT_w + b
                ostage = outpool.tile([128, SUP * 128], mybir.dt.float32, tag="ostage")
                for wi, w in enumerate(wins):
                    rh = rhpool.tile([128, 128], mybir.dt.float32, tag="rh")
                    nc.vector.tensor_copy(out=rh[:], in_=psbank[w][:])
                    mm = psopool.tile([D, 128], mybir.dt.float32, tag="po")
                    nc.tensor.matmul(
                        out=mm[:], lhsT=wt_t[:], rhs=rh[:], start=True, stop=True
                    )
                    nc.scalar.activation(
                        out=ostage[:, wi * 128 : (wi + 1) * 128],
                        in_=mm[:],
                        func=mybir.ActivationFunctionType.Identity,
                        bias=b_t[:, 0:1],
                        scale=1.0,
                    )
                nc.sync.dma_start(
                    out=outT[:, wins[0] * 128 : (wins[-1] + 1) * 128],
                    in_=ostage[:, : len(wins) * 128],
                )

    nc.finalize()
    return nc


def _preprocess(x, edge_index, W, b):
    """Host-side sharding: build per-core token tables + global schedule."""
    row = np.asarray(edge_index[0], dtype=np.int64)
    col = np.asarray(edge_index[1], dtype=np.int64)
    deg = (np.bincount(col, minlength=N) + 1).astype(np.float32)
    dinv = deg**-0.5  # float32, deg >= 1 always

    gam = (dinv[col] * dinv[row]).astype(np.float32)

    core = row // NPC
    lrow = row - core * NPC
    w = lrow // 128
    drel = (lrow % 128).astype(np.float32)
    S = w // SUP
    beta = col // BANK
    crel = (col - beta * BANK).astype(np.int16)

    # sort tokens by (core, S, beta, w)
    order = np.lexsort((w, beta, S, core))
    core_s = core[order]
    S_s = S[order]
    beta_s = beta[order]
    w_s = w[order]
    drel_s = drel[order]
    crel_s = crel[order]
    gam_s = gam[order]

    gid = ((core_s * NSUP + S_s) * NBANKS + beta_s) * NW + w_s
    NG = CORES * NSUP * NBANKS * NW
    counts = np.bincount(gid, minlength=NG).reshape(CORES, NSUP, NBANKS, NW)
    tiles = (counts + 127) // 128
    tiles_g = tiles.max(axis=0)  # [NSUP, NBANKS, NW]
    for s in range(NSUP):
        mask = np.zeros(NW, dtype=bool)
        mask[s * SUP : min((s + 1) * SUP, NW)] = True
        tiles_g[s, :, ~mask] = 0

    sup_windows = [list(range(s * SUP, min((s + 1) * SUP, NW))) for s in range(NSUP)]

    base = np.zeros((NSUP, NBANKS, NW), dtype=np.int64)
    cur = 0
    for s in range(NSUP):
        for bb in range(NBANKS):
            for ww in sup_windows[s]:
                base[s, bb, ww] = cur
                cur += 128 * int(tiles_g[s, bb, ww])
    T_total = cur // 128
    NTOK = cur

    Wt = np.ascontiguousarray(np.asarray(W, dtype=np.float32).T)
    bv = np.asarray(b, dtype=np.float32)[:, None].copy()
    xf = np.ascontiguousarray(np.asarray(x, dtype=np.float32))

    gid_full = core_s * (NSUP * NBANKS * NW) + (S_s * NBANKS + beta_s) * NW + w_s
    uniq, first_idx, cnt = np.unique(gid_full, return_index=True, return_counts=True)
    rank = np.arange(len(gid_full)) - np.repeat(first_idx, cnt)
    pos = base[S_s, beta_s, w_s] + rank

    core_bounds = np.searchsorted(core_s, np.arange(CORES + 1))
    in_maps = []
    for k in range(CORES):
        lo, hi = core_bounds[k], core_bounds[k + 1]
        idx16 = np.zeros(NTOK, dtype=np.int16)
        dr = np.zeros(NTOK, dtype=np.float32)
        gm = np.zeros(NTOK, dtype=np.float32)
        p = pos[lo:hi]
        idx16[p] = crel_s[lo:hi]
        dr[p] = drel_s[lo:hi]
        gm[p] = gam_s[lo:hi]
        idx_tile = np.tile(idx16.reshape(-1, 16).T, (8, 1))  # [128, NTOK//16]
        dr_t = np.ascontiguousarray(dr.reshape(T_total, 128).T)
        gm_t = np.ascontiguousarray(gm.reshape(T_total, 128).T)

        # self tables: gamma_self[p, w] = dinv[core row]^2 (0 beyond NPC)
        gs = np.zeros(NW * 128, dtype=np.float32)
        rows = np.arange(NPC) + k * NPC
        gs[:NPC] = dinv[rows] * dinv[rows]
        gs_t = np.ascontiguousarray(gs.reshape(NW, 128).T)
        # compact per-core x rows for self loads, padded to NW*128
        xs = np.zeros((NW * 128, D), dtype=np.float32)
        xs[:NPC] = xf[k * NPC : (k + 1) * NPC]
        in_maps.append(
            {
                "xt": xf,
                "idxs": idx_tile,
                "destrel": dr_t,
                "gamma": gm_t,
                "gself": gs_t,
                "xself": xs,
                "wt": Wt,
                "bvec": bv,
            }
        )

    return tiles_g, sup_windows, in_maps


_CACHE = {}


def kernel(x, edge_index, W, b, _want_trace=False):
    from concourse.bass_utils import run_bass_kernel_spmd

    tiles_g, sup_windows, in_maps = _preprocess(x, edge_index, W, b)
    key = tiles_g.tobytes()
    if key not in _CACHE:
        _CACHE[key] = _build_bass(tiles_g, sup_windows)
    nc = _CACHE[key]

    kwargs = {}
    if _want_trace:
        kwargs = dict(trace=True, trace_cores=list(range(CORES)))
    res = run_bass_kernel_spmd(nc, in_maps, core_ids=list(range(CORES)), **kwargs)

    out = np.empty((N, D), dtype=np.float32)
    for k in range(CORES):
        out[k * NPC : (k + 1) * NPC] = res.results[k]["outT"][:, :NPC].T
    if _want_trace:
        return out, res
    return out



# revision 2
# speedup vs baseline: 1.4455x; 1.4455x over previous
"""GCN message-passing kernel for Trainium2 (8 NeuronCores, SPMD).

out = (D^-1/2 (A+I) D^-1/2 X) W^T + b   for a random graph with
N=100000 nodes, E=1600000 edges, 128 channels.

Strategy (per core; destinations sharded 12500 nodes/core):
- Host pre-scales x by dinv (y = dinv * x, bf16). Every edge becomes a
  "token"; aggregation for a window of 128 destinations is
  aggT[ch, dst] = sum_tokens y[src]^T * onehot,
  onehot[e, d] = (destrel[e] == d), computed as bf16 matmuls
  accumulated in fp32 PSUM (lhsT = gathered message tile, rhs = one-hot).
  Padding tokens carry destrel=255 so the one-hot row is all-zero.
- Gathers use dma_gather (SWDGE, int16 indices, 256B bf16 rows) against
  4 banked views of y (32768 rows each); calls round-robin over 4 SWDGE
  queues so descriptor generation runs on all 8 GpSimd cores in parallel.
- Self-loops skip the gather: each window's own y rows load with a plain
  sequential DMA and enter the same one-hot matmul path (constant one-hot).
- Token order: [super of 6 windows][source bank][window][128-token tiles],
  padded so all 8 cores share one instruction stream.
- Finalize per window: Z^T = W^T @ aggT (bf16 x bf16 -> fp32), written as
  outT[128, 12544] per core; host transposes, applies dinv[dst] and bias.
"""

import sys

sys.path.insert(0, "/opt/trn_rl_repo")
import numpy as np

N = 100000
D = 128
CORES = 8
NPC = N // CORES  # 12500 dests per core
NW = (NPC + 127) // 128  # 98 windows per core
SUP = 6  # windows per super (PSUM accumulator banks: 6 + 2 for out matmul)
NSUP = (NW + SUP - 1) // SUP  # 17 supers
BANK = 32768
NBANKS = (N + BANK - 1) // BANK  # 4
CHUNK = 16  # gather-call size in 128-token tiles


def _build_bass(tiles, sup_windows):
    """Build the SPMD Bass program.

    tiles: int array [NSUP, NBANKS, NW] - tile count per group (global).
    sup_windows: list of per-super window lists.
    """
    import os

    import concourse.mybir as mybir
    import concourse.tile as tile
    from concourse import bacc

    lim_sup = int(os.environ.get("K_LIMIT_SUPERS", NSUP))
    T_total = int(tiles.sum())
    NTOK = 128 * T_total
    BF16 = mybir.dt.bfloat16

    nc = bacc.Bacc(None, target_bir_lowering=False, num_swdge_queues=4)
    xt = nc.dram_tensor("xt", [N, D], BF16, kind="ExternalInput")
    idxs = nc.dram_tensor("idxs", [128, NTOK // 16], mybir.dt.int16, kind="ExternalInput")
    destrel = nc.dram_tensor("destrel", [128, T_total], BF16, kind="ExternalInput")
    wt = nc.dram_tensor("wt", [D, D], BF16, kind="ExternalInput")
    outT = nc.dram_tensor("outT", [D, NW * 128], mybir.dt.float32, kind="ExternalOutput")

    xviews = [xt[b * BANK : min((b + 1) * BANK, N), :] for b in range(NBANKS)]

    # per-core compact y rows for self-loop loads (sequential DMA)
    xself_in = nc.dram_tensor("xself", [NW * 128, D], BF16, kind="ExternalInput")

    gq = [0]
    OHK = 16  # one-hot batch, in tiles
    with tile.TileContext(nc) as tc:
        with (
            tc.tile_pool(name="const", bufs=1) as cpool,
            tc.tile_pool(name="meta", bufs=1) as mpool,
            tc.tile_pool(name="gp", bufs=8) as gpool,
            tc.tile_pool(name="ohp", bufs=4) as ohpool,
            tc.tile_pool(name="selfp", bufs=3) as selfpool,
            tc.tile_pool(name="rhp", bufs=3) as rhpool,
            tc.tile_pool(name="outp", bufs=2) as outpool,
            tc.tile_pool(name="idxp", bufs=8) as idxpool,
            tc.tile_pool(name="ps", bufs=1, space="PSUM") as pspool,
            tc.tile_pool(name="pso", bufs=2, space="PSUM") as psopool,
        ):
            wt_t = cpool.tile([D, D], BF16)
            nc.sync.dma_start(out=wt_t[:], in_=wt[:])
            iota_f = cpool.tile([128, 128], mybir.dt.float32)
            nc.gpsimd.iota(
                iota_f[:], pattern=[[1, 128]], base=0, channel_multiplier=0,
                allow_small_or_imprecise_dtypes=True,
            )
            iota_t = cpool.tile([128, 128], BF16)
            nc.vector.tensor_copy(out=iota_t[:], in_=iota_f[:])
            pidx_f = cpool.tile([128, 1], mybir.dt.float32)
            nc.gpsimd.iota(
                pidx_f[:], pattern=[[1, 1]], base=0, channel_multiplier=1,
                allow_small_or_imprecise_dtypes=True,
            )
            pidx_t = cpool.tile([128, 1], BF16)
            nc.vector.tensor_copy(out=pidx_t[:], in_=pidx_f[:])
            # constant self one-hot: (iota == p)
            selfbase_t = cpool.tile([128, 128], BF16)
            nc.vector.tensor_tensor(
                out=selfbase_t[:],
                in0=iota_t[:],
                in1=pidx_t[:, 0:1].to_broadcast([128, 128]),
                op=mybir.AluOpType.is_equal,
            )
            destrel_t = mpool.tile([128, T_total], BF16)
            nc.sync.dma_start(out=destrel_t[:], in_=destrel[:])

            oh_batches = {}

            def oh_for(gt):
                bnum = gt // OHK
                if bnum not in oh_batches:
                    t0 = bnum * OHK
                    k = min(OHK, T_total - t0)
                    ohb = ohpool.tile([128, OHK, 128], BF16, tag="ohb")
                    nc.vector.tensor_tensor(
                        out=ohb[:, :k, :],
                        in0=iota_t[:, None, :].to_broadcast([128, k, 128]),
                        in1=destrel_t[:, t0 : t0 + k, None].to_broadcast([128, k, 128]),
                        op=mybir.AluOpType.is_equal,
                    )
                    oh_batches[bnum] = ohb
                    for old in list(oh_batches):
                        if old < bnum - 2:
                            del oh_batches[old]
                return oh_batches[bnum][:, gt % OHK, :]

            tile_cursor = 0  # global tile index in token order
            for S in range(NSUP):
                if S >= lim_sup:
                    break
                wins = sup_windows[S]
                psbank = {}
                mm_done = {w: 0 for w in wins}
                mm_total = {
                    w: 1 + int(sum(tiles[S, bb, w] for bb in range(NBANKS)))
                    for w in wins
                }
                for b in range(NBANKS):
                    region_tiles = int(sum(tiles[S, b, w] for w in wins))
                    chunk_tiles = []  # (start_tile_global, len, sbuf_tile)
                    c0 = 0
                    while c0 < region_tiles:
                        clen = min(CHUNK, region_tiles - c0)
                        gtile = gpool.tile([128, CHUNK, D], BF16, tag="g")
                        itile = idxpool.tile([128, CHUNK * 8], mybir.dt.int16, tag="ix")
                        gstart = tile_cursor + c0
                        nc.sync.dma_start(
                            out=itile[:, : clen * 8],
                            in_=idxs[:, gstart * 8 : (gstart + clen) * 8],
                        )
                        nc.gpsimd.dma_gather(
                            gtile[:, :clen, :],
                            xviews[b],
                            itile[:, : clen * 8],
                            128 * clen,
                            128 * clen,
                            D,
                            elem_step=D,
                            single_packet=False,
                            queue_num=gq[0] % 4,
                        )
                        gq[0] += 1
                        chunk_tiles.append((gstart, clen, gtile))
                        c0 += clen

                    def chunk_for(gt):
                        for cs, cl, ct in chunk_tiles:
                            if cs <= gt < cs + cl:
                                return ct, gt - cs
                        raise AssertionError

                    t_local = tile_cursor
                    for wi, w in enumerate(wins):
                        nt = int(tiles[S, b, w])
                        if b == 0:
                            # self-loop tile first: sequential y rows
                            ps = pspool.tile(
                                [128, 128], mybir.dt.float32, tag=f"psw{w % SUP}",
                                name=f"psw{S}_{w % SUP}",
                            )
                            psbank[w] = ps
                            xs = selfpool.tile([128, D], BF16, tag="xs")
                            nc.sync.dma_start(
                                out=xs[:], in_=xself_in[w * 128 : (w + 1) * 128, :]
                            )
                            nc.tensor.matmul(
                                out=ps[:],
                                lhsT=xs[:],
                                rhs=selfbase_t[:],
                                start=True,
                                stop=(mm_total[w] == 1),
                                skip_group_check=True,
                            )
                            mm_done[w] = 1
                        for t in range(nt):
                            gt = t_local + t
                            ctile, ccol = chunk_for(gt)
                            nc.tensor.matmul(
                                out=psbank[w][:],
                                lhsT=ctile[:, ccol, :],
                                rhs=oh_for(gt),
                                start=False,
                                stop=(mm_done[w] == mm_total[w] - 1),
                                skip_group_check=True,
                            )
                            mm_done[w] += 1
                        t_local += nt
                    tile_cursor += region_tiles

                # finalize super: outT_w = W^T @ aggT_w
                ostage = outpool.tile([128, SUP * 128], mybir.dt.float32, tag="ostage")
                for wi, w in enumerate(wins):
                    rh = rhpool.tile([128, 128], BF16, tag="rh")
                    nc.vector.tensor_copy(out=rh[:], in_=psbank[w][:])
                    mm = psopool.tile([D, 128], mybir.dt.float32, tag="po")
                    nc.tensor.matmul(
                        out=mm[:], lhsT=wt_t[:], rhs=rh[:], start=True, stop=True
                    )
                    nc.scalar.activation(
                        out=ostage[:, wi * 128 : (wi + 1) * 128],
                        in_=mm[:],
                        func=mybir.ActivationFunctionType.Identity,
                        scale=1.0,
                    )
                nc.sync.dma_start(
                    out=outT[:, wins[0] * 128 : (wins[-1] + 1) * 128],
                    in_=ostage[:, : len(wins) * 128],
                )

    nc.finalize()
    return nc


def _preprocess(x, edge_index, W, b):
    """Host-side sharding: build per-core token tables + global schedule."""
    import ml_dtypes

    row = np.asarray(edge_index[0], dtype=np.int64)
    col = np.asarray(edge_index[1], dtype=np.int64)
    deg = (np.bincount(col, minlength=N) + 1).astype(np.float32)
    dinv = deg**-0.5  # float32, deg >= 1 always

    core = row // NPC
    lrow = row - core * NPC
    w = lrow // 128
    drel = (lrow % 128).astype(np.float32)
    S = w // SUP
    beta = col // BANK
    crel = (col - beta * BANK).astype(np.int16)

    # sort tokens by (core, S, beta, w)
    order = np.lexsort((w, beta, S, core))
    core_s = core[order]
    S_s = S[order]
    beta_s = beta[order]
    w_s = w[order]
    drel_s = drel[order]
    crel_s = crel[order]

    gid = ((core_s * NSUP + S_s) * NBANKS + beta_s) * NW + w_s
    NG = CORES * NSUP * NBANKS * NW
    counts = np.bincount(gid, minlength=NG).reshape(CORES, NSUP, NBANKS, NW)
    tiles = (counts + 127) // 128
    tiles_g = tiles.max(axis=0)  # [NSUP, NBANKS, NW]
    for s in range(NSUP):
        mask = np.zeros(NW, dtype=bool)
        mask[s * SUP : min((s + 1) * SUP, NW)] = True
        tiles_g[s, :, ~mask] = 0

    sup_windows = [list(range(s * SUP, min((s + 1) * SUP, NW))) for s in range(NSUP)]

    base = np.zeros((NSUP, NBANKS, NW), dtype=np.int64)
    cur = 0
    for s in range(NSUP):
        for bb in range(NBANKS):
            for ww in sup_windows[s]:
                base[s, bb, ww] = cur
                cur += 128 * int(tiles_g[s, bb, ww])
    T_total = cur // 128
    NTOK = cur

    # pre-scale x by dinv; bf16 message rows
    yf = (np.asarray(x, dtype=np.float32) * dinv[:, None]).astype(ml_dtypes.bfloat16)
    Wt = np.ascontiguousarray(np.asarray(W, dtype=np.float32).T).astype(
        ml_dtypes.bfloat16
    )

    gid_full = core_s * (NSUP * NBANKS * NW) + (S_s * NBANKS + beta_s) * NW + w_s
    uniq, first_idx, cnt = np.unique(gid_full, return_index=True, return_counts=True)
    rank = np.arange(len(gid_full)) - np.repeat(first_idx, cnt)
    pos = base[S_s, beta_s, w_s] + rank

    core_bounds = np.searchsorted(core_s, np.arange(CORES + 1))
    in_maps = []
    for k in range(CORES):
        lo, hi = core_bounds[k], core_bounds[k + 1]
        idx16 = np.zeros(NTOK, dtype=np.int16)
        # padding tokens: destrel=255 never matches iota 0..127 -> zero one-hot
        dr = np.full(NTOK, 255.0, dtype=np.float32)
        p = pos[lo:hi]
        idx16[p] = crel_s[lo:hi]
        dr[p] = drel_s[lo:hi]
        idx_tile = np.tile(idx16.reshape(-1, 16).T, (8, 1))  # [128, NTOK//16]
        dr_t = np.ascontiguousarray(dr.reshape(T_total, 128).T).astype(
            ml_dtypes.bfloat16
        )

        # compact per-core y rows for self loads, padded to NW*128
        xs = np.zeros((NW * 128, D), dtype=ml_dtypes.bfloat16)
        xs[:NPC] = yf[k * NPC : (k + 1) * NPC]
        in_maps.append(
            {
                "xt": yf,
                "idxs": idx_tile,
                "destrel": dr_t,
                "xself": xs,
                "wt": Wt,
            }
        )

    return tiles_g, sup_windows, in_maps, dinv


_CACHE = {}


def kernel(x, edge_index, W, b, _want_trace=False):
    from concourse.bass_utils import run_bass_kernel_spmd

    tiles_g, sup_windows, in_maps, dinv = _preprocess(x, edge_index, W, b)
    key = tiles_g.tobytes()
    if key not in _CACHE:
        _CACHE[key] = _build_bass(tiles_g, sup_windows)
    nc = _CACHE[key]

    kwargs = {}
    if _want_trace:
        kwargs = dict(trace=True, trace_cores=list(range(CORES)))
    res = run_bass_kernel_spmd(nc, in_maps, core_ids=list(range(CORES)), **kwargs)

    bv = np.asarray(b, dtype=np.float32)[None, :]
    out = np.empty((N, D), dtype=np.float32)
    for k in range(CORES):
        z = res.results[k]["outT"][:, :NPC].T  # [NPC, D] = agg @ W^T
        out[k * NPC : (k + 1) * NPC] = (
            dinv[k * NPC : (k + 1) * NPC, None] * z + bv
        )
    if _want_trace:
        return out, res
    return out


# revision 5
# speedup vs baseline: 1.6181x; 1.1194x over previous
"""GCN message-passing kernel for Trainium2 (8 NeuronCores, SPMD).

out = (D^-1/2 (A+I) D^-1/2 X) W^T + b   for a random graph with
N=100000 nodes, E=1600000 edges, 128 channels.

Strategy (per core; destinations sharded 12500 nodes/core):
- Host pre-scales x by dinv (y = dinv * x, bf16). Every edge becomes a
  "token"; aggregation for a window of 128 destinations is
  aggT[ch, dst] = sum_tokens y[src]^T * onehot,
  onehot[e, d] = (destrel[e] == d), computed as bf16 matmuls
  accumulated in fp32 PSUM (lhsT = gathered message tile, rhs = one-hot).
  Padding tokens carry destrel=255 so the one-hot row is all-zero.
- Token tiles within each (super, bank) region are interleaved round-robin
  across the super's 6 windows so consecutive matmuls hit different PSUM
  banks (avoids same-bank accumulation serialization).
- Gathers use dma_gather (SWDGE, int16 indices, 256B bf16 rows) against
  4 banked views of y (32768 rows each); calls round-robin over 4 SWDGE
  queues. The full idx table and per-core self rows are preloaded into
  SBUF once, so gather calls never wait on metadata DMAs.
- Self-loops skip the gather: window w's own y rows come from the resident
  xself tile and enter the same one-hot matmul path (constant one-hot).
- Finalize per window: Z^T = W^T @ aggT (bf16 x bf16 -> fp32), written as
  outT[128, 12544] per core; host transposes, applies dinv[dst] and bias.
"""

import sys

sys.path.insert(0, "/opt/trn_rl_repo")
import numpy as np

N = 100000
D = 128
CORES = 8
NPC = N // CORES  # 12500 dests per core
NW = (NPC + 127) // 128  # 98 windows per core
SUP = 6  # windows per super (PSUM accumulator banks: 6 + 2 for out matmul)
NSUP = (NW + SUP - 1) // SUP  # 17 supers
BANK = 32768
NBANKS = (N + BANK - 1) // BANK  # 4
CHUNK = 16  # gather-call size in 128-token tiles


def _interleave(tiles, sup_windows):
    """Per (S,b): round-robin tile order across the super's windows.

    Returns dict (S,b) -> list of (w, t_in_window) in stream order.
    """
    order = {}
    for s, wins in enumerate(sup_windows):
        for b in range(NBANKS):
            lst = []
            for w in wins:
                for t in range(int(tiles[s, b, w])):
                    lst.append((t, w))
            lst.sort()
            order[(s, b)] = [(w, t) for (t, w) in lst]
    return order


def _build_bass(tiles, sup_windows):
    """Build the SPMD Bass program.

    tiles: int array [NSUP, NBANKS, NW] - tile count per group (global).
    sup_windows: list of per-super window lists.
    """
    import os

    import concourse.mybir as mybir
    import concourse.tile as tile
    from concourse import bacc

    lim_sup = int(os.environ.get("K_LIMIT_SUPERS", NSUP))
    T_total = int(tiles.sum())
    NTOK = 128 * T_total
    BF16 = mybir.dt.bfloat16
    order = _interleave(tiles, sup_windows)

    nc = bacc.Bacc(None, target_bir_lowering=False, num_swdge_queues=4)
    xt = nc.dram_tensor("xt", [N, D], BF16, kind="ExternalInput")
    idxs = nc.dram_tensor("idxs", [128, NTOK // 16], mybir.dt.int16, kind="ExternalInput")
    destrel = nc.dram_tensor("destrel", [128, T_total], BF16, kind="ExternalInput")
    wt = nc.dram_tensor("wt", [D, D], BF16, kind="ExternalInput")
    outT = nc.dram_tensor("outT", [D, NW * 128], mybir.dt.float32, kind="ExternalOutput")
    # per-core compact y rows for self-loop loads: [128, NW, D]
    xself_in = nc.dram_tensor("xself", [128, NW * D], BF16, kind="ExternalInput")

    xviews = [xt[b * BANK : min((b + 1) * BANK, N), :] for b in range(NBANKS)]

    gq = [0]
    OHK = 16  # one-hot batch, in tiles
    with tile.TileContext(nc) as tc:
        with (
            tc.tile_pool(name="const", bufs=1) as cpool,
            tc.tile_pool(name="meta", bufs=1) as mpool,
            tc.tile_pool(name="gp", bufs=12) as gpool,
            tc.tile_pool(name="ohp", bufs=6) as ohpool,
            tc.tile_pool(name="rhp", bufs=3) as rhpool,
            tc.tile_pool(name="outp", bufs=2) as outpool,
            tc.tile_pool(name="ps", bufs=1, space="PSUM") as pspool,
            tc.tile_pool(name="pso", bufs=2, space="PSUM") as psopool,
        ):
            wt_t = cpool.tile([D, D], BF16)
            nc.sync.dma_start(out=wt_t[:], in_=wt[:])
            iota_f = cpool.tile([128, 128], mybir.dt.float32)
            nc.gpsimd.iota(
                iota_f[:], pattern=[[1, 128]], base=0, channel_multiplier=0,
                allow_small_or_imprecise_dtypes=True,
            )
            iota_t = cpool.tile([128, 128], BF16)
            nc.vector.tensor_copy(out=iota_t[:], in_=iota_f[:])
            pidx_f = cpool.tile([128, 1], mybir.dt.float32)
            nc.gpsimd.iota(
                pidx_f[:], pattern=[[1, 1]], base=0, channel_multiplier=1,
                allow_small_or_imprecise_dtypes=True,
            )
            pidx_t = cpool.tile([128, 1], BF16)
            nc.vector.tensor_copy(out=pidx_t[:], in_=pidx_f[:])
            # constant self one-hot: (iota == p)
            selfbase_t = cpool.tile([128, 128], BF16)
            nc.vector.tensor_tensor(
                out=selfbase_t[:],
                in0=iota_t[:],
                in1=pidx_t[:, 0:1].to_broadcast([128, 128]),
                op=mybir.AluOpType.is_equal,
            )
            # resident metadata: full idx table, destrel, self rows
            idx_all = mpool.tile([128, NTOK // 16], mybir.dt.int16)
            nc.sync.dma_start(out=idx_all[:], in_=idxs[:])
            destrel_t = mpool.tile([128, T_total], BF16)
            nc.sync.dma_start(out=destrel_t[:], in_=destrel[:])
            xself_t = mpool.tile([128, NW * D], BF16)
            nc.sync.dma_start(out=xself_t[:], in_=xself_in[:])

            oh_batches = {}

            def oh_for(gt):
                bnum = gt // OHK
                if bnum not in oh_batches:
                    t0 = bnum * OHK
                    k = min(OHK, T_total - t0)
                    ohb = ohpool.tile([128, OHK, 128], BF16, tag="ohb")
                    nc.vector.tensor_tensor(
                        out=ohb[:, :k, :],
                        in0=iota_t[:, None, :].to_broadcast([128, k, 128]),
                        in1=destrel_t[:, t0 : t0 + k, None].to_broadcast([128, k, 128]),
                        op=mybir.AluOpType.is_equal,
                    )
                    oh_batches[bnum] = ohb
                    for old in list(oh_batches):
                        if old < bnum - 3:
                            del oh_batches[old]
                return oh_batches[bnum][:, gt % OHK, :]

            tile_cursor = 0  # global tile index in token order
            for S in range(NSUP):
                if S >= lim_sup:
                    break
                wins = sup_windows[S]
                psbank = {}
                mm_done = {w: 0 for w in wins}
                mm_total = {
                    w: 1 + int(sum(tiles[S, bb, w] for bb in range(NBANKS)))
                    for w in wins
                }
                for b in range(NBANKS):
                    region_tiles = int(sum(tiles[S, b, w] for w in wins))
                    chunk_tiles = []  # (start_tile_global, len, sbuf_tile)
                    c0 = 0
                    while c0 < region_tiles:
                        clen = min(CHUNK, region_tiles - c0)
                        gtile = gpool.tile([128, CHUNK, D], BF16, tag="g")
                        gstart = tile_cursor + c0
                        nc.gpsimd.dma_gather(
                            gtile[:, :clen, :],
                            xviews[b],
                            idx_all[:, gstart * 8 : (gstart + clen) * 8],
                            128 * clen,
                            128 * clen,
                            D,
                            elem_step=D,
                            single_packet=False,
                            queue_num=gq[0] % 4,
                        )
                        gq[0] += 1
                        chunk_tiles.append((gstart, clen, gtile))
                        c0 += clen

                    def chunk_for(gt):
                        for cs, cl, ct in chunk_tiles:
                            if cs <= gt < cs + cl:
                                return ct, gt - cs
                        raise AssertionError

                    if b == 0:
                        # self-loop tiles first: resident y rows, banks 0..5
                        for w in wins:
                            ps = pspool.tile(
                                [128, 128], mybir.dt.float32, tag=f"psw{w % SUP}",
                                name=f"psw{S}_{w % SUP}",
                            )
                            psbank[w] = ps
                            nc.tensor.matmul(
                                out=ps[:],
                                lhsT=xself_t[:, w * D : (w + 1) * D],
                                rhs=selfbase_t[:],
                                start=True,
                                stop=(mm_total[w] == 1),
                                skip_group_check=True,
                            )
                            mm_done[w] = 1
                    for slot, (w, t) in enumerate(order[(S, b)]):
                        gt = tile_cursor + slot
                        ctile, ccol = chunk_for(gt)
                        nc.tensor.matmul(
                            out=psbank[w][:],
                            lhsT=ctile[:, ccol, :],
                            rhs=oh_for(gt),
                            start=False,
                            stop=(mm_done[w] == mm_total[w] - 1),
                            skip_group_check=True,
                        )
                        mm_done[w] += 1
                    tile_cursor += region_tiles

                # finalize super: outT_w = W^T @ aggT_w
                ostage = outpool.tile([128, SUP * 128], mybir.dt.float32, tag="ostage")
                for wi, w in enumerate(wins):
                    rh = rhpool.tile([128, 128], BF16, tag="rh")
                    nc.scalar.activation(
                        out=rh[:],
                        in_=psbank[w][:],
                        func=mybir.ActivationFunctionType.Identity,
                        scale=1.0,
                    )
                    mm = psopool.tile([D, 128], mybir.dt.float32, tag="po")
                    nc.tensor.matmul(
                        out=mm[:], lhsT=wt_t[:], rhs=rh[:], start=True, stop=True
                    )
                    nc.scalar.activation(
                        out=ostage[:, wi * 128 : (wi + 1) * 128],
                        in_=mm[:],
                        func=mybir.ActivationFunctionType.Identity,
                        scale=1.0,
                    )
                nc.sync.dma_start(
                    out=outT[:, wins[0] * 128 : (wins[-1] + 1) * 128],
                    in_=ostage[:, : len(wins) * 128],
                )

    nc.finalize()
    return nc


def _preprocess(x, edge_index, W, b):
    """Host-side sharding: build per-core token tables + global schedule."""
    import ml_dtypes

    row = np.asarray(edge_index[0], dtype=np.int64)
    col = np.asarray(edge_index[1], dtype=np.int64)
    deg = (np.bincount(col, minlength=N) + 1).astype(np.float32)
    dinv = deg**-0.5  # float32, deg >= 1 always

    core = row // NPC
    lrow = row - core * NPC
    w = lrow // 128
    drel = (lrow % 128).astype(np.float32)
    S = w // SUP
    beta = col // BANK
    crel = (col - beta * BANK).astype(np.int16)

    # sort tokens by (core, S, beta, w)
    order = np.lexsort((w, beta, S, core))
    core_s = core[order]
    S_s = S[order]
    beta_s = beta[order]
    w_s = w[order]
    drel_s = drel[order]
    crel_s = crel[order]

    gid = ((core_s * NSUP + S_s) * NBANKS + beta_s) * NW + w_s
    NG = CORES * NSUP * NBANKS * NW
    counts = np.bincount(gid, minlength=NG).reshape(CORES, NSUP, NBANKS, NW)
    tiles = (counts + 127) // 128
    tiles_g = tiles.max(axis=0)  # [NSUP, NBANKS, NW]
    for s in range(NSUP):
        mask = np.zeros(NW, dtype=bool)
        mask[s * SUP : min((s + 1) * SUP, NW)] = True
        tiles_g[s, :, ~mask] = 0

    sup_windows = [list(range(s * SUP, min((s + 1) * SUP, NW))) for s in range(NSUP)]
    ilv = _interleave(tiles_g, sup_windows)

    # tile slot (global stream index) for each (s, b, w, t_in_window)
    slot_of = {}
    cur = 0
    region_base = {}
    for s in range(NSUP):
        for bb in range(NBANKS):
            region_base[(s, bb)] = cur
            for k, (ww, t) in enumerate(ilv[(s, bb)]):
                slot_of[(s, bb, ww, t)] = cur + k
            cur += len(ilv[(s, bb)])
    T_total = cur
    NTOK = cur * 128

    # base token position for each (s,b,w): token rank r in window-group
    # lands at slot_of[(s,b,w,r//128)]*128 + r%128
    # Vectorize: build arrays indexed by group gid3 = (s, b, w)
    slot_arr = np.zeros((NSUP, NBANKS, NW, int(tiles_g.max()) + 1), dtype=np.int64)
    for (s, bb, ww, t), sl in slot_of.items():
        slot_arr[s, bb, ww, t] = sl

    # pre-scale x by dinv; bf16 message rows
    yf = (np.asarray(x, dtype=np.float32) * dinv[:, None]).astype(ml_dtypes.bfloat16)
    Wt = np.ascontiguousarray(np.asarray(W, dtype=np.float32).T).astype(
        ml_dtypes.bfloat16
    )

    gid_full = core_s * (NSUP * NBANKS * NW) + (S_s * NBANKS + beta_s) * NW + w_s
    uniq, first_idx, cnt = np.unique(gid_full, return_index=True, return_counts=True)
    rank = np.arange(len(gid_full)) - np.repeat(first_idx, cnt)
    pos = slot_arr[S_s, beta_s, w_s, rank // 128] * 128 + rank % 128

    core_bounds = np.searchsorted(core_s, np.arange(CORES + 1))
    in_maps = []
    for k in range(CORES):
        lo, hi = core_bounds[k], core_bounds[k + 1]
        idx16 = np.zeros(NTOK, dtype=np.int16)
        # padding tokens: destrel=255 never matches iota 0..127 -> zero one-hot
        dr = np.full(NTOK, 255.0, dtype=np.float32)
        p = pos[lo:hi]
        idx16[p] = crel_s[lo:hi]
        dr[p] = drel_s[lo:hi]
        idx_tile = np.tile(idx16.reshape(-1, 16).T, (8, 1))  # [128, NTOK//16]
        dr_t = np.ascontiguousarray(dr.reshape(T_total, 128).T).astype(
            ml_dtypes.bfloat16
        )

        # compact per-core y rows for self loads: [128, NW, D] (partition = row%128)
        xs = np.zeros((NW * 128, D), dtype=ml_dtypes.bfloat16)
        xs[:NPC] = yf[k * NPC : (k + 1) * NPC]
        xs_t = np.ascontiguousarray(
            xs.reshape(NW, 128, D).transpose(1, 0, 2)
        ).reshape(128, NW * D)
        in_maps.append(
            {
                "xt": yf,
                "idxs": idx_tile,
                "destrel": dr_t,
                "xself": xs_t,
                "wt": Wt,
            }
        )

    return tiles_g, sup_windows, in_maps, dinv


_CACHE = {}


def kernel(x, edge_index, W, b, _want_trace=False):
    from concourse.bass_utils import run_bass_kernel_spmd

    tiles_g, sup_windows, in_maps, dinv = _preprocess(x, edge_index, W, b)
    key = tiles_g.tobytes()
    if key not in _CACHE:
        _CACHE[key] = _build_bass(tiles_g, sup_windows)
    nc = _CACHE[key]

    kwargs = {}
    if _want_trace:
        kwargs = dict(trace=True, trace_cores=list(range(CORES)))
    res = run_bass_kernel_spmd(nc, in_maps, core_ids=list(range(CORES)), **kwargs)

    bv = np.asarray(b, dtype=np.float32)[None, :]
    out = np.empty((N, D), dtype=np.float32)
    for k in range(CORES):
        z = res.results[k]["outT"][:, :NPC].T  # [NPC, D] = agg @ W^T
        out[k * NPC : (k + 1) * NPC] = (
            dinv[k * NPC : (k + 1) * NPC, None] * z + bv
        )
    if _want_trace:
        return out, res
    return out


# revision 10
# speedup vs baseline: 1.6357x; 1.0109x over previous
"""GCN message-passing kernel for Trainium2 (8 NeuronCores, SPMD).

out = (D^-1/2 (A+I) D^-1/2 X) W^T + b   for a random graph with
N=100000 nodes, E=1600000 edges, 128 channels.

Strategy (per core; destinations sharded 12500 nodes/core):
- Host pre-scales x by dinv (y = dinv * x, bf16). Every edge becomes a
  "token"; aggregation for a window of 128 destinations is
  aggT[ch, dst] = sum_tokens y[src]^T * onehot,
  onehot[e, d] = (destrel[e] == d), computed as bf16 matmuls
  accumulated in fp32 PSUM (lhsT = gathered message tile, rhs = one-hot).
- Tokens are packed CONTIGUOUSLY per (super, src-bank) region (window
  capacity = max count over cores, no per-window tile rounding); a tile
  straddling a window boundary is consumed by one matmul per window it
  touches ("uses"), each with its own one-hot column built from a per-use
  destrel table (fp16; non-members hold 512 which never matches iota 0..127).
  This cuts SWDGE gather descriptors ~9% - the kernel is descriptor-
  generation-bound (~300 descs/us/core through the GpSimd SWDGE queues).
- One dma_gather call per region (~68 calls) round-robins the 4 SWDGE
  queues; the full idx table, destrel table and self rows are preloaded
  into SBUF so gathers never wait on metadata.
- Self-loops skip the gather: window w's own y rows come from the resident
  xself tile and enter the same one-hot matmul path (constant one-hot).
- Finalize per super: Z^T = W^T @ aggT for 3 windows per matmul
  (rhs [128, 384]); host transposes, applies dinv[dst] and bias.
"""

import sys

sys.path.insert(0, "/opt/trn_rl_repo")
import numpy as np

N = 100000
D = 128
CORES = 8
NPC = N // CORES  # 12500 dests per core
NW = (NPC + 127) // 128  # 98 windows per core
SUP = 6  # windows per super (PSUM accumulator banks: 6 + 2 for out matmul)
NSUP = (NW + SUP - 1) // SUP  # 17 supers
BANK = 32768
NBANKS = (N + BANK - 1) // BANK  # 4
OHK = 16  # one-hot batch, in uses
PAD_DR = 512.0  # fp16-exact, never matches iota 0..127


def _schedule(win_cap):
    """Build the contiguous packing schedule from per-window capacities.

    win_cap: [NSUP, NBANKS, NW] int - max-over-cores token count per group.
    Returns (regions, uses, R, mm_total, T_total, U_total):
      regions: dict (s,b) -> dict with 'tile_base', 'ntiles',
               'uses': list of (w, t_local, u_global, off_in_region)
      mm_total: [NW] total matmuls per window (1 self + uses)
    """
    sup_windows = [list(range(s * SUP, min((s + 1) * SUP, NW))) for s in range(NSUP)]
    regions = {}
    mm_total = np.ones(NW, dtype=np.int64)
    tile_base = 0
    u_global = 0
    for s in range(NSUP):
        wins = sup_windows[s]
        for b in range(NBANKS):
            off = 0
            uses = []
            offs = {}
            for w in wins:
                cap = int(win_cap[s, b, w])
                offs[w] = off
                if cap > 0:
                    t_lo = off // 128
                    t_hi = (off + cap - 1) // 128
                    for t in range(t_lo, t_hi + 1):
                        uses.append((w, t, u_global, off))
                        u_global += 1
                        mm_total[w] += 1
                off += cap
            ntiles = (off + 127) // 128
            regions[(s, b)] = {
                "tile_base": tile_base,
                "ntiles": ntiles,
                "uses": uses,
                "offs": offs,
            }
            tile_base += ntiles
    return sup_windows, regions, mm_total, tile_base, u_global


def _build_bass(win_cap):
    """Build the SPMD Bass program from the packing schedule."""
    import concourse.mybir as mybir
    import concourse.tile as tile
    from concourse import bacc

    sup_windows, regions, mm_total, T_total, U_total = _schedule(win_cap)
    NTOK = 128 * T_total
    RMAX = max(r["ntiles"] for r in regions.values())
    BF16 = mybir.dt.bfloat16
    FP16 = mybir.dt.float16

    nc = bacc.Bacc(None, target_bir_lowering=False, num_swdge_queues=4)
    xt = nc.dram_tensor("xt", [N, D], BF16, kind="ExternalInput")
    idxs = nc.dram_tensor("idxs", [128, NTOK // 16], mybir.dt.int16, kind="ExternalInput")
    destrel = nc.dram_tensor("destrel", [128, U_total], FP16, kind="ExternalInput")
    wt = nc.dram_tensor("wt", [D, D], BF16, kind="ExternalInput")
    outT = nc.dram_tensor("outT", [D, NW * 128], mybir.dt.float32, kind="ExternalOutput")
    xself_in = nc.dram_tensor("xself", [128, NW * D], BF16, kind="ExternalInput")

    xviews = [xt[b * BANK : min((b + 1) * BANK, N), :] for b in range(NBANKS)]

    gq = [0]
    with tile.TileContext(nc) as tc:
        with (
            tc.tile_pool(name="const", bufs=1) as cpool,
            tc.tile_pool(name="meta", bufs=1) as mpool,
            tc.tile_pool(name="gp", bufs=8) as gpool,
            tc.tile_pool(name="ohp", bufs=6) as ohpool,
            tc.tile_pool(name="rhp", bufs=3) as rhpool,
            tc.tile_pool(name="outp", bufs=2) as outpool,
            tc.tile_pool(name="ps", bufs=1, space="PSUM") as pspool,
            tc.tile_pool(name="pso", bufs=2, space="PSUM") as psopool,
        ):
            # idx table first: gathers depend only on it
            idx_all = mpool.tile([128, NTOK // 16], mybir.dt.int16)
            nc.sync.dma_start(out=idx_all[:], in_=idxs[:])
            destrel_t = mpool.tile([128, U_total], FP16)
            nc.sync.dma_start(out=destrel_t[:], in_=destrel[:])
            xself_t = mpool.tile([128, NW * D], BF16)
            nc.sync.dma_start(out=xself_t[:], in_=xself_in[:])
            wt_t = cpool.tile([D, D], BF16)
            nc.sync.dma_start(out=wt_t[:], in_=wt[:])

            iota_f = cpool.tile([128, 128], mybir.dt.float32)
            nc.gpsimd.iota(
                iota_f[:], pattern=[[1, 128]], base=0, channel_multiplier=0,
                allow_small_or_imprecise_dtypes=True,
            )
            iota_t = cpool.tile([128, 128], FP16)
            nc.vector.tensor_copy(out=iota_t[:], in_=iota_f[:])
            pidx_f = cpool.tile([128, 1], mybir.dt.float32)
            nc.gpsimd.iota(
                pidx_f[:], pattern=[[1, 1]], base=0, channel_multiplier=1,
                allow_small_or_imprecise_dtypes=True,
            )
            pidx_t = cpool.tile([128, 1], FP16)
            nc.vector.tensor_copy(out=pidx_t[:], in_=pidx_f[:])
            selfbase_t = cpool.tile([128, 128], BF16)
            nc.vector.tensor_tensor(
                out=selfbase_t[:],
                in0=iota_t[:],
                in1=pidx_t[:, 0:1].to_broadcast([128, 128]),
                op=mybir.AluOpType.is_equal,
            )

            oh_batches = {}

            def oh_for(u):
                bnum = u // OHK
                if bnum not in oh_batches:
                    u0 = bnum * OHK
                    k = min(OHK, U_total - u0)
                    ohb = ohpool.tile([128, OHK, 128], BF16, tag="ohb")
                    nc.vector.tensor_tensor(
                        out=ohb[:, :k, :],
                        in0=iota_t[:, None, :].to_broadcast([128, k, 128]),
                        in1=destrel_t[:, u0 : u0 + k, None].to_broadcast([128, k, 128]),
                        op=mybir.AluOpType.is_equal,
                    )
                    oh_batches[bnum] = ohb
                    for old in list(oh_batches):
                        if old < bnum - 3:
                            del oh_batches[old]
                return oh_batches[bnum][:, u % OHK, :]

            for S in range(NSUP):
                wins = sup_windows[S]
                psbank = {}
                mm_done = {w: 0 for w in wins}
                for b in range(NBANKS):
                    reg = regions[(S, b)]
                    ntiles = reg["ntiles"]
                    gtile = None
                    if ntiles > 0:
                        gtile = gpool.tile([128, RMAX, D], BF16, tag="g")
                        gs = reg["tile_base"]
                        nc.gpsimd.dma_gather(
                            gtile[:, :ntiles, :],
                            xviews[b],
                            idx_all[:, gs * 8 : (gs + ntiles) * 8],
                            128 * ntiles,
                            128 * ntiles,
                            D,
                            elem_step=D,
                            single_packet=False,
                            queue_num=gq[0] % 4,
                        )
                        gq[0] += 1
                    if b == 0:
                        for w in wins:
                            ps = pspool.tile(
                                [128, 128], mybir.dt.float32, tag=f"psw{w % SUP}",
                                name=f"psw{S}_{w % SUP}",
                            )
                            psbank[w] = ps
                            nc.tensor.matmul(
                                out=ps[:],
                                lhsT=xself_t[:, w * D : (w + 1) * D],
                                rhs=selfbase_t[:],
                                start=True,
                                stop=(mm_total[w] == 1),
                                skip_group_check=True,
                            )
                            mm_done[w] = 1
                    for w, t, u, off in reg["uses"]:
                        nc.tensor.matmul(
                            out=psbank[w][:],
                            lhsT=gtile[:, t, :],
                            rhs=oh_for(u),
                            start=False,
                            stop=(mm_done[w] == mm_total[w] - 1),
                            skip_group_check=True,
                        )
                        mm_done[w] += 1

                # finalize super: outT = W^T @ aggT, 3 windows per matmul
                nwin = len(wins)
                ostage = outpool.tile([128, SUP * 128], mybir.dt.float32, tag="ostage")
                for g0 in range(0, nwin, 3):
                    gw = wins[g0 : g0 + 3]
                    rh = rhpool.tile([128, 3 * 128], BF16, tag="rh")
                    for j, w in enumerate(gw):
                        nc.scalar.activation(
                            out=rh[:, j * 128 : (j + 1) * 128],
                            in_=psbank[w][:],
                            func=mybir.ActivationFunctionType.Identity,
                            scale=1.0,
                        )
                    mm = psopool.tile([D, 3 * 128], mybir.dt.float32, tag="po")
                    k = len(gw) * 128
                    nc.tensor.matmul(
                        out=mm[:, :k], lhsT=wt_t[:], rhs=rh[:, :k],
                        start=True, stop=True,
                    )
                    nc.scalar.activation(
                        out=ostage[:, g0 * 128 : g0 * 128 + k],
                        in_=mm[:, :k],
                        func=mybir.ActivationFunctionType.Identity,
                        scale=1.0,
                    )
                nc.sync.dma_start(
                    out=outT[:, wins[0] * 128 : (wins[-1] + 1) * 128],
                    in_=ostage[:, : nwin * 128],
                )

    nc.finalize()
    return nc


def _preprocess(x, edge_index, W, b):
    """Host-side sharding: build per-core token tables + global schedule."""
    import ml_dtypes

    row = np.asarray(edge_index[0], dtype=np.int64)
    col = np.asarray(edge_index[1], dtype=np.int64)
    deg = (np.bincount(col, minlength=N) + 1).astype(np.float32)
    dinv = deg**-0.5  # float32, deg >= 1 always

    core = row // NPC
    lrow = row - core * NPC
    w = lrow // 128
    drel = (lrow % 128).astype(np.float32)
    S = w // SUP
    beta = col // BANK
    crel = (col - beta * BANK).astype(np.int16)

    order = np.lexsort((w, beta, S, core))
    core_s = core[order]
    S_s = S[order]
    beta_s = beta[order]
    w_s = w[order]
    drel_s = drel[order]
    crel_s = crel[order]

    gid = ((core_s * NSUP + S_s) * NBANKS + beta_s) * NW + w_s
    NG = CORES * NSUP * NBANKS * NW
    counts = np.bincount(gid, minlength=NG).reshape(CORES, NSUP, NBANKS, NW)
    win_cap = counts.max(axis=0)  # [NSUP, NBANKS, NW]

    sup_windows, regions, mm_total, T_total, U_total = _schedule(win_cap)
    NTOK = 128 * T_total

    # token placement: global token index for (s,b,w,rank)
    tok_base = np.zeros((NSUP, NBANKS, NW), dtype=np.int64)
    for (s, b_), reg in regions.items():
        for ww, off in reg["offs"].items():
            tok_base[s, b_, ww] = reg["tile_base"] * 128 + off
    # use index lookup: (s,b,w,t_local) -> u
    use_of = {}
    for (s, b_), reg in regions.items():
        for ww, t, u, off in reg["uses"]:
            use_of[(s, b_, ww, t)] = u

    yf = (np.asarray(x, dtype=np.float32) * dinv[:, None]).astype(ml_dtypes.bfloat16)
    Wt = np.ascontiguousarray(np.asarray(W, dtype=np.float32).T).astype(
        ml_dtypes.bfloat16
    )

    gid_full = core_s * (NSUP * NBANKS * NW) + (S_s * NBANKS + beta_s) * NW + w_s
    uniq, first_idx, cnt = np.unique(gid_full, return_index=True, return_counts=True)
    rank = np.arange(len(gid_full)) - np.repeat(first_idx, cnt)
    pos = tok_base[S_s, beta_s, w_s] + rank  # global token position

    # per-token use index: tile t_local = (off+rank)//128 - tile_base... compute
    reg_tile_base = np.zeros((NSUP, NBANKS), dtype=np.int64)
    for (s, b_), reg in regions.items():
        reg_tile_base[s, b_] = reg["tile_base"]
    t_local = pos // 128 - reg_tile_base[S_s, beta_s]
    u_arr = np.empty(len(pos), dtype=np.int64)
    # vectorized-ish lookup via dict (1.6M entries, loop in C via map)
    keys = list(zip(S_s.tolist(), beta_s.tolist(), w_s.tolist(), t_local.tolist()))
    u_arr[:] = [use_of[k] for k in keys]

    core_bounds = np.searchsorted(core_s, np.arange(CORES + 1))
    in_maps = []
    for k in range(CORES):
        lo, hi = core_bounds[k], core_bounds[k + 1]
        idx16 = np.zeros(NTOK, dtype=np.int16)
        dr = np.full((128, U_total), PAD_DR, dtype=np.float32)
        p = pos[lo:hi]
        idx16[p] = crel_s[lo:hi]
        dr[p % 128, u_arr[lo:hi]] = drel_s[lo:hi]
        idx_tile = np.tile(idx16.reshape(-1, 16).T, (8, 1))  # [128, NTOK//16]
        dr_t = dr.astype(ml_dtypes.float16 if hasattr(ml_dtypes, "float16") else np.float16)

        xs = np.zeros((NW * 128, D), dtype=ml_dtypes.bfloat16)
        xs[:NPC] = yf[k * NPC : (k + 1) * NPC]
        xs_t = np.ascontiguousarray(
            xs.reshape(NW, 128, D).transpose(1, 0, 2)
        ).reshape(128, NW * D)
        in_maps.append(
            {
                "xt": yf,
                "idxs": idx_tile,
                "destrel": dr_t,
                "xself": xs_t,
                "wt": Wt,
            }
        )

    return win_cap, in_maps, dinv


_CACHE = {}


def kernel(x, edge_index, W, b, _want_trace=False):
    from concourse.bass_utils import run_bass_kernel_spmd

    win_cap, in_maps, dinv = _preprocess(x, edge_index, W, b)
    key = win_cap.tobytes()
    if key not in _CACHE:
        _CACHE[key] = _build_bass(win_cap)
    nc = _CACHE[key]

    kwargs = {}
    if _want_trace:
        kwargs = dict(trace=True, trace_cores=list(range(CORES)))
    res = run_bass_kernel_spmd(nc, in_maps, core_ids=list(range(CORES)), **kwargs)

    bv = np.asarray(b, dtype=np.float32)[None, :]
    out = np.empty((N, D), dtype=np.float32)
    for k in range(CORES):
        z = res.results[k]["outT"][:, :NPC].T  # [NPC, D] = agg @ W^T
        out[k * NPC : (k + 1) * NPC] = (
            dinv[k * NPC : (k + 1) * NPC, None] * z + bv
        )
    if _want_trace:
        return out, res
    return out


# revision 11
# speedup vs baseline: 2.2024x; 1.3464x over previous
"""GCN message-passing kernel for Trainium2 (8 NeuronCores, SPMD).

out = (D^-1/2 (A+I) D^-1/2 X) W^T + b   for a random graph with
N=100000 nodes, E=1600000 edges, 128 channels.

Strategy (per core; destinations sharded 12500 nodes/core):
- Host pre-scales x by dinv (y = dinv * x, bf16). Every edge becomes a
  "token"; aggregation for a window of 128 destinations is
  aggT[ch, dst] = sum_tokens y[src]^T * onehot,
  onehot[e, d] = (destrel[e] == d), computed as bf16 matmuls
  accumulated in fp32 PSUM (lhsT = gathered message tile, rhs = one-hot).
- Tokens are packed CONTIGUOUSLY per (super, src-bank) region (window
  capacity = max count over cores, no per-window tile rounding); a tile
  straddling a window boundary is consumed by one matmul per window it
  touches ("uses"), each with its own one-hot column built from a per-use
  destrel table (fp16; non-members hold 512 which never matches iota 0..127).
  This cuts SWDGE gather descriptors ~9% - the kernel is descriptor-
  generation-bound (~300 descs/us/core through the GpSimd SWDGE queues).
- One dma_gather call per region (~68 calls) round-robins the 4 SWDGE
  queues; the full idx table, destrel table and self rows are preloaded
  into SBUF so gathers never wait on metadata.
- Self-loops skip the gather: window w's own y rows come from the resident
  xself tile and enter the same one-hot matmul path (constant one-hot).
- Finalize per super: Z^T = W^T @ aggT for 3 windows per matmul
  (rhs [128, 384]); host transposes, applies dinv[dst] and bias.
"""

import sys

sys.path.insert(0, "/opt/trn_rl_repo")
import numpy as np

N = 100000
D = 128
CORES = 8
NPC = N // CORES  # 12500 dests per core
NW = (NPC + 127) // 128  # 98 windows per core
SUP = 6  # windows per super (PSUM accumulator banks: 6 + 2 for out matmul)
NSUP = (NW + SUP - 1) // SUP  # 17 supers
BANK = 32768
NBANKS = (N + BANK - 1) // BANK  # 4
OHK = 16  # one-hot batch, in uses
PAD_DR = 512.0  # fp16-exact, never matches iota 0..127


def _schedule(win_cap):
    """Build the contiguous packing schedule from per-window capacities.

    win_cap: [NSUP, NBANKS, NW] int - max-over-cores token count per group.
    Returns (regions, uses, R, mm_total, T_total, U_total):
      regions: dict (s,b) -> dict with 'tile_base', 'ntiles',
               'uses': list of (w, t_local, u_global, off_in_region)
      mm_total: [NW] total matmuls per window (1 self + uses)
    """
    sup_windows = [list(range(s * SUP, min((s + 1) * SUP, NW))) for s in range(NSUP)]
    regions = {}
    mm_total = np.ones(NW, dtype=np.int64)
    tile_base = 0
    u_global = 0
    for s in range(NSUP):
        wins = sup_windows[s]
        for b in range(NBANKS):
            off = 0
            uses = []
            offs = {}
            for w in wins:
                cap = int(win_cap[s, b, w])
                offs[w] = off
                if cap > 0:
                    t_lo = off // 128
                    t_hi = (off + cap - 1) // 128
                    for t in range(t_lo, t_hi + 1):
                        uses.append((w, t, u_global, off))
                        u_global += 1
                        mm_total[w] += 1
                off += cap
            ntiles = (off + 127) // 128
            regions[(s, b)] = {
                "tile_base": tile_base,
                "ntiles": ntiles,
                "uses": uses,
                "offs": offs,
            }
            tile_base += ntiles
    return sup_windows, regions, mm_total, tile_base, u_global


def _build_bass(win_cap):
    """Build the SPMD Bass program from the packing schedule."""
    import concourse.mybir as mybir
    import concourse.tile as tile
    from concourse import bacc

    sup_windows, regions, mm_total, T_total, U_total = _schedule(win_cap)
    NTOK = 128 * T_total
    RMAX = max(r["ntiles"] for r in regions.values())
    BF16 = mybir.dt.bfloat16
    FP16 = mybir.dt.float16

    nc = bacc.Bacc(None, target_bir_lowering=False, num_swdge_queues=4)
    xt = nc.dram_tensor("xt", [N, D], BF16, kind="ExternalInput")
    idxs = nc.dram_tensor("idxs", [128, NTOK // 16], mybir.dt.int16, kind="ExternalInput")
    destrel = nc.dram_tensor("destrel", [128, U_total], FP16, kind="ExternalInput")
    wt = nc.dram_tensor("wt", [D, D], BF16, kind="ExternalInput")
    outT = nc.dram_tensor("outT", [D, NW * 128], mybir.dt.float32, kind="ExternalOutput")
    xself_in = nc.dram_tensor("xself", [128, NW * D], BF16, kind="ExternalInput")

    xviews = [xt[b * BANK : min((b + 1) * BANK, N), :] for b in range(NBANKS)]

    gq = [0]
    with tile.TileContext(nc) as tc:
        with (
            tc.tile_pool(name="const", bufs=1) as cpool,
            tc.tile_pool(name="meta", bufs=1) as mpool,
            tc.tile_pool(name="gp", bufs=8) as gpool,
            tc.tile_pool(name="ohp", bufs=6) as ohpool,
            tc.tile_pool(name="rhp", bufs=3) as rhpool,
            tc.tile_pool(name="outp", bufs=2) as outpool,
            tc.tile_pool(name="ps", bufs=1, space="PSUM") as pspool,
            tc.tile_pool(name="pso", bufs=2, space="PSUM") as psopool,
        ):
            # idx table first: gathers depend only on it
            idx_all = mpool.tile([128, NTOK // 16], mybir.dt.int16)
            nc.sync.dma_start(out=idx_all[:], in_=idxs[:])
            destrel_t = mpool.tile([128, U_total], FP16)
            nc.sync.dma_start(out=destrel_t[:], in_=destrel[:])
            xself_t = mpool.tile([128, NW * D], BF16)
            nc.sync.dma_start(out=xself_t[:], in_=xself_in[:])
            wt_t = cpool.tile([D, D], BF16)
            nc.sync.dma_start(out=wt_t[:], in_=wt[:])

            iota_f = cpool.tile([128, 128], mybir.dt.float32)
            nc.gpsimd.iota(
                iota_f[:], pattern=[[1, 128]], base=0, channel_multiplier=0,
                allow_small_or_imprecise_dtypes=True,
            )
            iota_t = cpool.tile([128, 128], FP16)
            nc.vector.tensor_copy(out=iota_t[:], in_=iota_f[:])
            pidx_f = cpool.tile([128, 1], mybir.dt.float32)
            nc.gpsimd.iota(
                pidx_f[:], pattern=[[1, 1]], base=0, channel_multiplier=1,
                allow_small_or_imprecise_dtypes=True,
            )
            pidx_t = cpool.tile([128, 1], FP16)
            nc.vector.tensor_copy(out=pidx_t[:], in_=pidx_f[:])
            selfbase_t = cpool.tile([128, 128], BF16)
            nc.vector.tensor_tensor(
                out=selfbase_t[:],
                in0=iota_t[:],
                in1=pidx_t[:, 0:1].to_broadcast([128, 128]),
                op=mybir.AluOpType.is_equal,
            )

            oh_batches = {}

            def oh_for(u):
                bnum = u // OHK
                if bnum not in oh_batches:
                    u0 = bnum * OHK
                    k = min(OHK, U_total - u0)
                    ohb = ohpool.tile([128, OHK, 128], BF16, tag="ohb")
                    nc.vector.tensor_tensor(
                        out=ohb[:, :k, :],
                        in0=iota_t[:, None, :].to_broadcast([128, k, 128]),
                        in1=destrel_t[:, u0 : u0 + k, None].to_broadcast([128, k, 128]),
                        op=mybir.AluOpType.is_equal,
                    )
                    oh_batches[bnum] = ohb
                    for old in list(oh_batches):
                        if old < bnum - 3:
                            del oh_batches[old]
                return oh_batches[bnum][:, u % OHK, :]

            for S in range(NSUP):
                wins = sup_windows[S]
                psbank = {}
                mm_done = {w: 0 for w in wins}
                for b in range(NBANKS):
                    reg = regions[(S, b)]
                    ntiles = reg["ntiles"]
                    gtile = None
                    if ntiles > 0:
                        gtile = gpool.tile([128, RMAX, D], BF16, tag="g")
                        gs = reg["tile_base"]
                        nc.gpsimd.dma_gather(
                            gtile[:, :ntiles, :],
                            xviews[b],
                            idx_all[:, gs * 8 : (gs + ntiles) * 8],
                            128 * ntiles,
                            128 * ntiles,
                            D,
                            elem_step=D,
                            single_packet=False,
                            queue_num=gq[0] % 4,
                        )
                        gq[0] += 1
                    if b == 0:
                        for w in wins:
                            ps = pspool.tile(
                                [128, 128], mybir.dt.float32, tag=f"psw{w % SUP}",
                                name=f"psw{S}_{w % SUP}",
                            )
                            psbank[w] = ps
                            nc.tensor.matmul(
                                out=ps[:],
                                lhsT=xself_t[:, w * D : (w + 1) * D],
                                rhs=selfbase_t[:],
                                start=True,
                                stop=(mm_total[w] == 1),
                                skip_group_check=True,
                            )
                            mm_done[w] = 1
                    for w, t, u, off in reg["uses"]:
                        nc.tensor.matmul(
                            out=psbank[w][:],
                            lhsT=gtile[:, t, :],
                            rhs=oh_for(u),
                            start=False,
                            stop=(mm_done[w] == mm_total[w] - 1),
                            skip_group_check=True,
                        )
                        mm_done[w] += 1

                # finalize super: outT = W^T @ aggT, 3 windows per matmul
                nwin = len(wins)
                ostage = outpool.tile([128, SUP * 128], mybir.dt.float32, tag="ostage")
                for g0 in range(0, nwin, 3):
                    gw = wins[g0 : g0 + 3]
                    rh = rhpool.tile([128, 3 * 128], BF16, tag="rh")
                    for j, w in enumerate(gw):
                        nc.scalar.activation(
                            out=rh[:, j * 128 : (j + 1) * 128],
                            in_=psbank[w][:],
                            func=mybir.ActivationFunctionType.Identity,
                            scale=1.0,
                        )
                    mm = psopool.tile([D, 3 * 128], mybir.dt.float32, tag="po")
                    k = len(gw) * 128
                    nc.tensor.matmul(
                        out=mm[:, :k], lhsT=wt_t[:], rhs=rh[:, :k],
                        start=True, stop=True,
                    )
                    nc.scalar.activation(
                        out=ostage[:, g0 * 128 : g0 * 128 + k],
                        in_=mm[:, :k],
                        func=mybir.ActivationFunctionType.Identity,
                        scale=1.0,
                    )
                nc.sync.dma_start(
                    out=outT[:, wins[0] * 128 : (wins[-1] + 1) * 128],
                    in_=ostage[:, : nwin * 128],
                )

    nc.finalize()
    return nc


def _preprocess(x, edge_index, W, b):
    """Host-side sharding: build per-core token tables + global schedule."""
    import ml_dtypes

    row = np.asarray(edge_index[0], dtype=np.int64)
    col = np.asarray(edge_index[1], dtype=np.int64)
    deg = (np.bincount(col, minlength=N) + 1).astype(np.float32)
    dinv = deg**-0.5  # float32, deg >= 1 always

    core = row // NPC
    lrow = row - core * NPC
    w = lrow // 128
    drel = (lrow % 128).astype(np.float32)
    S = w // SUP
    beta = col // BANK
    crel = (col - beta * BANK).astype(np.int16)

    order = np.lexsort((w, beta, S, core))
    core_s = core[order]
    S_s = S[order]
    beta_s = beta[order]
    w_s = w[order]
    drel_s = drel[order]
    crel_s = crel[order]

    gid = ((core_s * NSUP + S_s) * NBANKS + beta_s) * NW + w_s
    NG = CORES * NSUP * NBANKS * NW
    counts = np.bincount(gid, minlength=NG).reshape(CORES, NSUP, NBANKS, NW)
    win_cap = counts.max(axis=0)  # [NSUP, NBANKS, NW]

    sup_windows, regions, mm_total, T_total, U_total = _schedule(win_cap)
    NTOK = 128 * T_total

    # token placement: global token index for (s,b,w,rank)
    tok_base = np.zeros((NSUP, NBANKS, NW), dtype=np.int64)
    for (s, b_), reg in regions.items():
        for ww, off in reg["offs"].items():
            tok_base[s, b_, ww] = reg["tile_base"] * 128 + off
    # use index lookup: (s,b,w,t_local) -> u
    use_of = {}
    for (s, b_), reg in regions.items():
        for ww, t, u, off in reg["uses"]:
            use_of[(s, b_, ww, t)] = u

    yf = (np.asarray(x, dtype=np.float32) * dinv[:, None]).astype(ml_dtypes.bfloat16)
    Wt = np.ascontiguousarray(np.asarray(W, dtype=np.float32).T).astype(
        ml_dtypes.bfloat16
    )

    gid_full = core_s * (NSUP * NBANKS * NW) + (S_s * NBANKS + beta_s) * NW + w_s
    uniq, first_idx, cnt = np.unique(gid_full, return_index=True, return_counts=True)
    rank = np.arange(len(gid_full)) - np.repeat(first_idx, cnt)
    pos = tok_base[S_s, beta_s, w_s] + rank  # global token position

    # per-token use index: tile t_local = (off+rank)//128 - tile_base... compute
    reg_tile_base = np.zeros((NSUP, NBANKS), dtype=np.int64)
    for (s, b_), reg in regions.items():
        reg_tile_base[s, b_] = reg["tile_base"]
    t_local = pos // 128 - reg_tile_base[S_s, beta_s]
    u_arr = np.empty(len(pos), dtype=np.int64)
    # vectorized-ish lookup via dict (1.6M entries, loop in C via map)
    keys = list(zip(S_s.tolist(), beta_s.tolist(), w_s.tolist(), t_local.tolist()))
    u_arr[:] = [use_of[k] for k in keys]

    core_bounds = np.searchsorted(core_s, np.arange(CORES + 1))
    in_maps = []
    for k in range(CORES):
        lo, hi = core_bounds[k], core_bounds[k + 1]
        idx16 = np.zeros(NTOK, dtype=np.int16)
        dr = np.full((128, U_total), PAD_DR, dtype=np.float32)
        p = pos[lo:hi]
        idx16[p] = crel_s[lo:hi]
        dr[p % 128, u_arr[lo:hi]] = drel_s[lo:hi]
        idx_tile = np.tile(idx16.reshape(-1, 16).T, (8, 1))  # [128, NTOK//16]
        dr_t = dr.astype(np.float16)

        xs = np.zeros((NW * 128, D), dtype=ml_dtypes.bfloat16)
        xs[:NPC] = yf[k * NPC : (k + 1) * NPC]
        xs_t = np.ascontiguousarray(
            xs.reshape(NW, 128, D).transpose(1, 0, 2)
        ).reshape(128, NW * D)
        in_maps.append(
            {
                "xt": yf,
                "idxs": idx_tile,
                "destrel": dr_t,
                "xself": xs_t,
                "wt": Wt,
            }
        )

    return win_cap, in_maps, dinv


_CACHE = {}


def kernel(x, edge_index, W, b, _want_trace=False):
    from concourse.bass_utils import run_bass_kernel_spmd

    win_cap, in_maps, dinv = _preprocess(x, edge_index, W, b)
    key = win_cap.tobytes()
    if key not in _CACHE:
        _CACHE[key] = _build_bass(win_cap)
    nc = _CACHE[key]

    kwargs = {}
    if _want_trace:
        kwargs = dict(trace=True, trace_cores=list(range(CORES)))
    res = run_bass_kernel_spmd(nc, in_maps, core_ids=list(range(CORES)), **kwargs)

    bv = np.asarray(b, dtype=np.float32)[None, :]
    out = np.empty((N, D), dtype=np.float32)
    for k in range(CORES):
        z = res.results[k]["outT"][:, :NPC].T  # [NPC, D] = agg @ W^T
        out[k * NPC : (k + 1) * NPC] = (
            dinv[k * NPC : (k + 1) * NPC, None] * z + bv
        )
    if _want_trace:
        return out, res
    return out
